# revision 7
# baseline (speedup 1.0000x reference)
"""TRN2 Bass kernel for nn_BilinearInteraction — scheme C (no-pad, 3-engine split).

out[b,k] = sum_{e,f} E[b,i,e] W[k,e,f] E[b,j,f], 780 pairs (i<j) lexicographic,
40 fields, e=f=32.

Per core (batch shard 1024 = 8 b-tiles of 128 on partitions):
- Stage 1 (PE): per i, matmul u[b,(j,f)] = E_i^T(32e x 128b).T @ W_i(32e x Nj*32)
  in <=16-slot sub-matmuls into 2-bank PSUM super-chunks. Contraction rows sit
  at 32*(i%4) (explicit tile_position) so consecutive i's run on different
  row-groups concurrently.
- Elementwise u*Ej split across engines (GPS_FRAC of elements on GpSimd
  directly from PSUM fp32; the rest: ScalarE evicts PSUM->fp16, VectorE
  multiplies at 2x).
- Reduce f 32->1: fp16 binary tree on VectorE per half (i<11 / i>=11),
  final level adds into fp32 output row.
- Output is already in k-order: host just reshapes.
"""

import numpy as np

import concourse.bass as bass
import concourse.mybir as mybir
import concourse.tile as tile
from concourse import bacc
from concourse.bass_utils import run_bass_kernel_spmd

NF = 40
E = 32
NPAIR = 780
BATCH = 8192
NCORES = 8
B_CORE = BATCH // NCORES
NBT = B_CORE // 128            # 8 b-tiles per core

GPS_FRAC = 0.37                # fraction of elements multiplied on GpSimd
SC = 64                        # super-chunk slots (4 PSUM banks)
MM = 16                        # slots per matmul (1 bank, 512 fp32)
HALF_I = 11                    # half A: i 0..10 (374 slots), B: 11..38 (406)

NI = [NF - 1 - i for i in range(NF - 1)]            # pairs per i
SBASE = np.concatenate([[0], np.cumsum(NI)]).astype(int)   # slot base per i
assert SBASE[NF - 1] == NPAIR
SA = int(SBASE[HALF_I])        # 374
SB = NPAIR - SA                # 406

# per-residue W packing offsets
POSR = {}
_rlen = [0, 0, 0, 0]
for _i in range(NF - 1):
    _r = _i % 4
    POSR[_i] = _rlen[_r]
    _rlen[_r] += NI[_i]
WL = max(_rlen)                # 210


def _pack_w(W):
    wp = np.zeros((128, WL, E), np.float32)
    for i in range(NF - 1):
        r = i % 4
        n = NI[i]
        k0 = SBASE[i]
        wp[32 * r:32 * r + 32, POSR[i]:POSR[i] + n, :] = (
            W[k0:k0 + n].transpose(1, 0, 2))
    return wp


def _pack_et(emb):
    # [NC, B, 10, 4, E] -> [NC, 4(r), E(e), 10(m), B]
    et = emb.reshape(NCORES, B_CORE, NF // 4, 4, E).transpose(0, 3, 4, 2, 1)
    return np.ascontiguousarray(et.reshape(NCORES, 128, NF // 4, B_CORE))


# ---------------- bass program ----------------
_CACHED = None


def _build():
    global _CACHED
    if _CACHED is not None:
        return _CACHED

    nc = bacc.Bacc("TRN2", target_bir_lowering=False, debug=False)
    f32 = mybir.dt.float32
    f16 = mybir.dt.float16

    et16_d = nc.dram_tensor("et16", [128, NF // 4, B_CORE], f16, kind="ExternalInput")
    wp_d = nc.dram_tensor("wp", [128, WL, E], f16, kind="ExternalInput")
    e16n_d = nc.dram_tensor("e16n", [NBT, 128, NF, E], f16, kind="ExternalInput")
    o_d = nc.dram_tensor("o", [NBT, 128, NPAIR], f32, kind="ExternalOutput")

    # build the static super-chunk schedule: (i, off, nj, engine)
    # engine: 0 = scalar-evict + vector-mul, 1 = gpsimd direct from PSUM
    sched = []
    tot = 0
    gps = 0
    for i in range(NF - 1):
        for off in range(0, NI[i], SC):
            nj = min(SC, NI[i] - off)
            ne = nj * E
            if gps + ne <= GPS_FRAC * (tot + ne):
                eng = 1
                gps += ne
            else:
                eng = 0
            tot += ne
            sched.append((i, off, nj, eng))

    with tile.TileContext(nc) as tc:
        with (
            tc.tile_pool(name="consts", bufs=1) as consts,
            tc.tile_pool(name="en", bufs=2) as en,
            tc.tile_pool(name="ue", bufs=4) as uep,
            tc.tile_pool(name="vpool", bufs=1) as vpool,
            tc.tile_pool(name="tree", bufs=2) as tree,
            tc.tile_pool(name="outs", bufs=2) as outs,
            tc.tile_pool(name="upsum", bufs=2, space="PSUM") as upsum,
        ):
            wp_sb = consts.tile([128, WL, E], f16)
            for s in range(0, WL, 53):
                e2 = min(s + 53, WL)
                nc.sync.dma_start(out=wp_sb[:, s:e2, :], in_=wp_d[:, s:e2, :])
            et16_sb = consts.tile([128, NF // 4, B_CORE], f16)
            for m in range(NF // 4):
                nc.sync.dma_start(out=et16_sb[:, m, :], in_=et16_d[:, m, :])

            for bt in range(NBT):
                bs = bass.ts(bt, 128)
                e16n = en.tile([128, NF, E], f16, tag="e16n")
                nc.sync.dma_start(out=e16n[:], in_=e16n_d[bt, :, :, :])
                obt = outs.tile([128, NPAIR], f32, tag="obt")

                for half in range(2):
                    if half == 0:
                        ilo, ihi, hbase, Sh = 0, HALF_I, 0, SA
                    else:
                        ilo, ihi, hbase, Sh = HALF_I, NF - 1, SA, SB
                    vh = vpool.tile([128, SB, E], f16, tag=f"v{half}")

                    for (i, off, nj, eng) in sched:
                        if not (ilo <= i < ihi):
                            continue
                        r = i % 4
                        j0 = i + 1 + off
                        sb0 = int(SBASE[i]) + off - hbase
                        u_ps = upsum.tile([128, SC, E], f32, tag="u")
                        for c0 in range(0, nj, MM):
                            n1 = min(MM, nj - c0)
                            nc.tensor.matmul(
                                u_ps[:, c0:c0 + n1, :],
                                et16_sb[32 * r:32 * r + 32, i // 4, bs],
                                wp_sb[32 * r:32 * r + 32,
                                      POSR[i] + off + c0:POSR[i] + off + c0 + n1, :],
                                start=True,
                                stop=True,
                                tile_position=(32 * r, 0),
                            )
                        ue = uep.tile([128, SC, E], f16, tag="ue")
                        nc.scalar.copy(out=ue[:, :nj, :], in_=u_ps[:, :nj, :])
                        meng = nc.gpsimd if eng == 1 else nc.vector
                        meng.tensor_mul(
                            vh[:, sb0:sb0 + nj, :],
                            ue[:, :nj, :],
                            e16n[:, j0:j0 + nj, :],
                        )

                    s1 = tree.tile([128, SB, 16], f16, tag="s1")
                    nc.vector.tensor_add(
                        s1[:, :Sh, :], vh[:, :Sh, 0:16], vh[:, :Sh, 16:32])
                    s2 = tree.tile([128, SB, 8], f16, tag="s2")
                    nc.vector.tensor_add(
                        s2[:, :Sh, :], s1[:, :Sh, 0:8], s1[:, :Sh, 8:16])
                    s3 = tree.tile([128, SB, 4], f16, tag="s3")
                    nc.vector.tensor_add(
                        s3[:, :Sh, :], s2[:, :Sh, 0:4], s2[:, :Sh, 4:8])
                    s4 = tree.tile([128, SB, 2], f16, tag="s4")
                    nc.vector.tensor_add(
                        s4[:, :Sh, :], s3[:, :Sh, 0:2], s3[:, :Sh, 2:4])
                    nc.vector.tensor_add(
                        obt[:, hbase:hbase + Sh],
                        s4[:, :Sh, 0],
                        s4[:, :Sh, 1],
                    )

                nc.sync.dma_start(out=o_d[bt, :, :], in_=obt[:])

    nc.compile()
    _CACHED = nc
    return nc


# ---------------- public entry ----------------
def _run(embeddings, W, **spmd_kwargs):
    embeddings = np.ascontiguousarray(np.asarray(embeddings, dtype=np.float32))
    W = np.ascontiguousarray(np.asarray(W, dtype=np.float32))

    et16 = _pack_et(embeddings).astype(np.float16)
    e16n = np.ascontiguousarray(
        embeddings.reshape(NCORES, NBT, 128, NF, E).astype(np.float16))
    wp = _pack_w(W).astype(np.float16)

    nc = _build()
    in_maps = [
        {"et16": et16[c], "wp": wp, "e16n": e16n[c]}
        for c in range(NCORES)
    ]
    res = run_bass_kernel_spmd(nc, in_maps, list(range(NCORES)), **spmd_kwargs)

    out = np.empty((BATCH, NPAIR), np.float32)
    for c in range(NCORES):
        out[c * B_CORE:(c + 1) * B_CORE] = res.results[c]["o"].reshape(B_CORE, NPAIR)
    return out, res


def kernel(embeddings, W):
    out, _ = _run(embeddings, W)
    return out


# revision 10
# speedup vs baseline: 1.0484x; 1.0484x over previous
"""TRN2 Bass kernel for nn_BilinearInteraction — scheme C (no-pad, 3-engine split).

out[b,k] = sum_{e,f} E[b,i,e] W[k,e,f] E[b,j,f], 780 pairs (i<j) lexicographic,
40 fields, e=f=32.

Per core (batch shard 1024 = 8 b-tiles of 128 on partitions):
- Stage 1 (PE): per i, matmul u[b,(j,f)] = E_i^T(32e x 128b).T @ W_i(32e x Nj*32)
  in <=16-slot sub-matmuls into 2-bank PSUM super-chunks. Contraction rows sit
  at 32*(i%4) (explicit tile_position) so consecutive i's run on different
  row-groups concurrently.
- Elementwise u*Ej split across engines (GPS_FRAC of elements on GpSimd
  directly from PSUM fp32; the rest: ScalarE evicts PSUM->fp16, VectorE
  multiplies at 2x).
- Reduce f 32->1: fp16 binary tree on VectorE per half (i<11 / i>=11),
  final level adds into fp32 output row.
- Output is already in k-order: host just reshapes.
"""

import numpy as np

import concourse.bass as bass
import concourse.mybir as mybir
import concourse.tile as tile
from concourse import bacc
from concourse.bass_utils import run_bass_kernel_spmd

NF = 40
E = 32
NPAIR = 780
BATCH = 8192
NCORES = 8
B_CORE = BATCH // NCORES
NBT = B_CORE // 128            # 8 b-tiles per core

GPS_FRAC = 0.37                # fraction of elements multiplied on GpSimd
SC = 32                        # super-chunk slots (2 PSUM banks)
MM = 16                        # slots per matmul (1 bank, 512 fp32)
HALF_I = 11                    # half A: i 0..10 (374 slots), B: 11..38 (406)

NI = [NF - 1 - i for i in range(NF - 1)]            # pairs per i
SBASE = np.concatenate([[0], np.cumsum(NI)]).astype(int)   # slot base per i
assert SBASE[NF - 1] == NPAIR
SA = int(SBASE[HALF_I])        # 374
SB = NPAIR - SA                # 406

# per-residue W packing offsets
POSR = {}
_rlen = [0, 0, 0, 0]
for _i in range(NF - 1):
    _r = _i % 4
    POSR[_i] = _rlen[_r]
    _rlen[_r] += NI[_i]
WL = max(_rlen)                # 210


def _pack_w(W):
    wp = np.zeros((128, WL, E), np.float32)
    for i in range(NF - 1):
        r = i % 4
        n = NI[i]
        k0 = SBASE[i]
        wp[32 * r:32 * r + 32, POSR[i]:POSR[i] + n, :] = (
            W[k0:k0 + n].transpose(1, 0, 2))
    return wp


def _pack_et(emb):
    # [NC, B, 10, 4, E] -> [NC, 4(r), E(e), 10(m), B]
    et = emb.reshape(NCORES, B_CORE, NF // 4, 4, E).transpose(0, 3, 4, 2, 1)
    return np.ascontiguousarray(et.reshape(NCORES, 128, NF // 4, B_CORE))


# ---------------- bass program ----------------
_CACHED = None


def _build():
    global _CACHED
    if _CACHED is not None:
        return _CACHED

    nc = bacc.Bacc("TRN2", target_bir_lowering=False, debug=False)
    f32 = mybir.dt.float32
    f16 = mybir.dt.float16

    et16_d = nc.dram_tensor("et16", [128, NF // 4, B_CORE], f16, kind="ExternalInput")
    wp_d = nc.dram_tensor("wp", [128, WL, E], f16, kind="ExternalInput")
    e16n_d = nc.dram_tensor("e16n", [NBT, 128, NF, E], f16, kind="ExternalInput")
    o_d = nc.dram_tensor("o", [NBT, 128, NPAIR], f32, kind="ExternalOutput")

    # build the static super-chunk schedule: (i, off, nj, engine)
    # engine: 0 = scalar-evict + vector-mul, 1 = gpsimd direct from PSUM
    sched = []
    tot = 0
    gps = 0
    for i in range(NF - 1):
        for off in range(0, NI[i], SC):
            nj = min(SC, NI[i] - off)
            ne = nj * E
            if gps + ne <= GPS_FRAC * (tot + ne):
                eng = 1
                gps += ne
            else:
                eng = 0
            tot += ne
            sched.append((i, off, nj, eng))

    with tile.TileContext(nc) as tc:
        with (
            tc.tile_pool(name="consts", bufs=1) as consts,
            tc.tile_pool(name="en", bufs=2) as en,
            tc.tile_pool(name="ue", bufs=8) as uep,
            tc.tile_pool(name="vpool", bufs=1) as vpool,
            tc.tile_pool(name="tree", bufs=2) as tree,
            tc.tile_pool(name="outs", bufs=2) as outs,
            tc.tile_pool(name="upsum", bufs=4, space="PSUM") as upsum,
        ):
            wp_sb = consts.tile([128, WL, E], f16)
            for s in range(0, WL, 53):
                e2 = min(s + 53, WL)
                nc.sync.dma_start(out=wp_sb[:, s:e2, :], in_=wp_d[:, s:e2, :])
            et16_sb = consts.tile([128, NF // 4, B_CORE], f16)
            for m in range(NF // 4):
                nc.sync.dma_start(out=et16_sb[:, m, :], in_=et16_d[:, m, :])

            for bt in range(NBT):
                bs = bass.ts(bt, 128)
                e16n = en.tile([128, NF, E], f16, tag="e16n")
                nc.sync.dma_start(out=e16n[:], in_=e16n_d[bt, :, :, :])
                obt = outs.tile([128, NPAIR], f32, tag="obt")

                for half in range(2):
                    if half == 0:
                        ilo, ihi, hbase, Sh = 0, HALF_I, 0, SA
                    else:
                        ilo, ihi, hbase, Sh = HALF_I, NF - 1, SA, SB
                    vh = vpool.tile([128, SB, E], f16, tag=f"v{half}")

                    for (i, off, nj, eng) in sched:
                        if not (ilo <= i < ihi):
                            continue
                        r = i % 4
                        j0 = i + 1 + off
                        sb0 = int(SBASE[i]) + off - hbase
                        u_ps = upsum.tile([128, SC, E], f32, tag="u")
                        for c0 in range(0, nj, MM):
                            n1 = min(MM, nj - c0)
                            nc.tensor.matmul(
                                u_ps[:, c0:c0 + n1, :],
                                et16_sb[32 * r:32 * r + 32, i // 4, bs],
                                wp_sb[32 * r:32 * r + 32,
                                      POSR[i] + off + c0:POSR[i] + off + c0 + n1, :],
                                start=True,
                                stop=True,
                                tile_position=(32 * r, 0),
                            )
                        ue = uep.tile([128, SC, E], f16, tag="ue")
                        nc.scalar.copy(out=ue[:, :nj, :], in_=u_ps[:, :nj, :])
                        meng = nc.gpsimd if eng == 1 else nc.vector
                        meng.tensor_mul(
                            vh[:, sb0:sb0 + nj, :],
                            ue[:, :nj, :],
                            e16n[:, j0:j0 + nj, :],
                        )

                    s1 = tree.tile([128, SB, 16], f16, tag="s1")
                    nc.vector.tensor_add(
                        s1[:, :Sh, :], vh[:, :Sh, 0:16], vh[:, :Sh, 16:32])
                    s2 = tree.tile([128, SB, 8], f16, tag="s2")
                    nc.vector.tensor_add(
                        s2[:, :Sh, :], s1[:, :Sh, 0:8], s1[:, :Sh, 8:16])
                    s3 = tree.tile([128, SB, 4], f16, tag="s3")
                    nc.vector.tensor_add(
                        s3[:, :Sh, :], s2[:, :Sh, 0:4], s2[:, :Sh, 4:8])
                    s4 = tree.tile([128, SB, 2], f16, tag="s4")
                    nc.vector.tensor_add(
                        s4[:, :Sh, :], s3[:, :Sh, 0:2], s3[:, :Sh, 2:4])
                    nc.vector.tensor_add(
                        obt[:, hbase:hbase + Sh],
                        s4[:, :Sh, 0],
                        s4[:, :Sh, 1],
                    )

                nc.sync.dma_start(out=o_d[bt, :, :], in_=obt[:])

    nc.compile()
    _CACHED = nc
    return nc


# ---------------- public entry ----------------
def _run(embeddings, W, **spmd_kwargs):
    embeddings = np.ascontiguousarray(np.asarray(embeddings, dtype=np.float32))
    W = np.ascontiguousarray(np.asarray(W, dtype=np.float32))

    et16 = _pack_et(embeddings).astype(np.float16)
    e16n = np.ascontiguousarray(
        embeddings.reshape(NCORES, NBT, 128, NF, E).astype(np.float16))
    wp = _pack_w(W).astype(np.float16)

    nc = _build()
    in_maps = [
        {"et16": et16[c], "wp": wp, "e16n": e16n[c]}
        for c in range(NCORES)
    ]
    res = run_bass_kernel_spmd(nc, in_maps, list(range(NCORES)), **spmd_kwargs)

    out = np.empty((BATCH, NPAIR), np.float32)
    for c in range(NCORES):
        out[c * B_CORE:(c + 1) * B_CORE] = res.results[c]["o"].reshape(B_CORE, NPAIR)
    return out, res


def kernel(embeddings, W):
    out, _ = _run(embeddings, W)
    return out
